# revision 22
# baseline (speedup 1.0000x reference)
"""DRQN forward on 8 TRN2 NeuronCores, data-parallel over batch.

Reference computation (per batch row b):
    x      = concat(obs[b], act[b])                       [544]
    y1     = relu(W1 @ x + b1)                            [512]
    gates  = W_ih @ y1 + b_ih + (W_hh @ h + b_hh)         [2048]
    i,f,g,o = split(gates, 4); i,f,o = sigmoid; g = tanh
    c_new  = f * c + i * g                                [512]
    h_new  = o * tanh(c_new)                              [512]
    q      = W2 @ h_new + b2                              [32]
Returns (q [B,32], h_new[-1], c_new[-1]).

Device strategy (per core, batch shard of 4096 rows, 4 tiles of 1024):
  - activations kept transposed [feature, batch] in SBUF; the one real
    transpose (x) uses the DMA X-bar (bf16, one 3D-out instruction per
    128-row subtile) after an f32->bf16 cast on GpSimd.
  - all matmuls bf16 with fp32 PSUM accumulation.
  - gate bias (b_ih + b_hh + W_hh @ h) is per-partition in this layout and
    fuses into the ScalarE sigmoid/tanh ops; computed once on-device via
    DVE elementwise + reduce against a broadcast h.
  - gate types processed g,i,f,o so the LSTM elementwise work (DVE) can
    start per hid-chunk as soon as its inputs' activations land.
  - q computed transposed [32, batch], X-bar'd back, cast f32, stored
    in natural layout.
"""

import sys

sys.path.insert(0, "/opt/trn_rl_repo")

from contextlib import ExitStack

import numpy as np
import ml_dtypes

import concourse.bass as bass
import concourse.tile as tile
from concourse import bacc, mybir
from concourse.bass_utils import run_bass_kernel_spmd

BF16 = mybir.dt.bfloat16
F32 = mybir.dt.float32
AF = mybir.ActivationFunctionType
OP = mybir.AluOpType

OBS, ACT, EMB, HID, NQ = 512, 32, 512, 512, 32
B = 32768
NCORES = 8
BC = B // NCORES          # rows per core
TILE = 1024               # batch rows per main-loop tile
NT = BC // TILE           # main-loop tiles per core
NU = TILE // 128          # 128-row subtiles per tile
XF = 640                  # padded input feature dim (544 -> 5*128)
NKX = XF // 128           # K chunks for MLP1
NKE = EMB // 128          # K chunks over EMB
NKH = HID // 128          # K chunks over HID

_COMPILED = None


def _build():
    nc = bacc.Bacc(
        "TRN2", target_bir_lowering=False, debug=False,
        enable_asserts=False, num_devices=NCORES,
    )

    obs_e = nc.dram_tensor("observation", [BC, OBS], F32, kind="ExternalInput").ap()
    act_e = nc.dram_tensor("prev_action", [BC, ACT], F32, kind="ExternalInput").ap()
    hpp_e = nc.dram_tensor("hpp", [128, NKH], F32, kind="ExternalInput").ap()
    w1t_e = nc.dram_tensor("w1t", [128, NKX, EMB], BF16, kind="ExternalInput").ap()
    wih_e = nc.dram_tensor("wih", [128, NKE, 4 * HID], BF16, kind="ExternalInput").ap()
    whht_e = nc.dram_tensor("whht", [128, NKH, 4 * HID], BF16, kind="ExternalInput").ap()
    w2t_e = nc.dram_tensor("w2t", [128, NKH, NQ], BF16, kind="ExternalInput").ap()
    b1_e = nc.dram_tensor("b1pp", [128, NKE], F32, kind="ExternalInput").ap()
    bsum_e = nc.dram_tensor("bsumpp", [128, 16], F32, kind="ExternalInput").ap()
    c_e = nc.dram_tensor("cpp", [128, NKH], F32, kind="ExternalInput").ap()
    b2_e = nc.dram_tensor("b2pp", [NQ, 1], F32, kind="ExternalInput").ap()

    q_o = nc.dram_tensor("q_out", [BC, NQ], F32, kind="ExternalOutput").ap()
    # h_new[-1] and c_new[-1], stored [128, chunk] (hc[p, c] = v[c*128+p]);
    # the host de-permutes. Avoids a 512x4B-descriptor DMA in the tail.
    hc_o = nc.dram_tensor("hc_out", [128, 2 * NKH], F32, kind="ExternalOutput").ap()

    with tile.TileContext(nc) as tc, ExitStack() as ctx:
        consts = ctx.enter_context(tc.tile_pool(name="consts", bufs=1))
        obs_p = ctx.enter_context(tc.tile_pool(name="obs", bufs=4))
        act_p = ctx.enter_context(tc.tile_pool(name="act", bufs=4))
        xbf_p = ctx.enter_context(tc.tile_pool(name="xbf", bufs=2))
        xt_p = ctx.enter_context(tc.tile_pool(name="xt", bufs=2))
        y1_p = ctx.enter_context(tc.tile_pool(name="y1", bufs=2))
        gate_p = ctx.enter_context(tc.tile_pool(name="gate", bufs=4))
        ig_p = ctx.enter_context(tc.tile_pool(name="ig", bufs=2))
        cn_p = ctx.enter_context(tc.tile_pool(name="cn", bufs=2))
        h_p = ctx.enter_context(tc.tile_pool(name="hsb", bufs=2))
        qt_p = ctx.enter_context(tc.tile_pool(name="qt", bufs=2))
        qtr_p = ctx.enter_context(tc.tile_pool(name="qtr", bufs=2))
        qf_p = ctx.enter_context(tc.tile_pool(name="qf", bufs=2))
        out_p = ctx.enter_context(tc.tile_pool(name="outs", bufs=1))

        y1ps = ctx.enter_context(tc.tile_pool(name="y1ps", bufs=2, space="PSUM"))
        gps = ctx.enter_context(tc.tile_pool(name="gps", bufs=2, space="PSUM"))
        qps = ctx.enter_context(tc.tile_pool(name="qps", bufs=1, space="PSUM"))

        # ---- x load + cast + X-bar transpose for one tile; one xbf tile
        # per 128-row subtile so each X-bar waits only on its own casts ----
        def load_x(t, cast_eng):
            b0 = t * TILE
            xt = xt_p.tile([128, NKX, TILE], BF16, tag="xt", name="xt")
            for u in range(NU):
                r0 = b0 + u * 128
                obs_r = obs_p.tile([128, OBS], F32, tag="obs", name="obs_r")
                nc.sync.dma_start(out=obs_r[:], in_=obs_e[r0 : r0 + 128, :])
                act_r = act_p.tile([128, ACT], F32, tag="act", name="act_r")
                nc.sync.dma_start(out=act_r[:], in_=act_e[r0 : r0 + 128, :])
                xbf = xbf_p.tile([128, XF], BF16, tag="xbf", name="xbf")
                cast_eng.tensor_copy(xbf[:, 0:OBS], obs_r[:])
                cast_eng.tensor_copy(xbf[:, OBS : OBS + ACT], act_r[:])
                cast_eng.memset(xbf[:, OBS + ACT : XF], 0.0)
                nc.sync.dma_start(
                    out=xt[:, :, u * 128 : (u + 1) * 128],
                    in_=xbf[:],
                    transpose=True,
                )
            return xt

        # ---- preamble inputs for gb first, then tile-0 inputs, then the rest
        hppf = consts.tile([128, NKH], F32)
        nc.sync.dma_start(out=hppf[:], in_=hpp_e[:])
        whht = consts.tile([128, NKH, 4 * HID], BF16)
        nc.sync.dma_start(out=whht[:], in_=whht_e[:])
        bsum = consts.tile([128, 16], F32)
        nc.sync.dma_start(out=bsum[:], in_=bsum_e[:])
        gb = consts.tile([128, 16], F32)

        # gb[j*128+p] = (W_hh @ h)[j*128+p] + (b_ih+b_hh), via 64 tiny N=1
        # matmuls on the (otherwise idle at the head) TensorE
        hT = consts.tile([128, NKH], BF16)
        nc.vector.tensor_copy(hT[:], hppf[:])
        gbp = y1ps.tile([128, 16], F32, tag="yp", name="gbp")
        for j in range(16):
            for k in range(NKH):
                nc.tensor.matmul(
                    gbp[:, j : j + 1],
                    whht[:, k, j * 128 : (j + 1) * 128],
                    hT[:, k : k + 1],
                    start=(k == 0),
                    stop=(k == NKH - 1),
                )
        nc.vector.tensor_add(gb[:], gbp[:], bsum[:])

        # tile-0 input DMAs + casts on DVE (GpSimd casts are ~4x slower and
        # tile 0 is on the critical path)
        xt_cur = load_x(0, nc.vector)

        # ---- remaining constants ----
        w1t = consts.tile([128, NKX, EMB], BF16)
        nc.sync.dma_start(out=w1t[:], in_=w1t_e[:])
        b1 = consts.tile([128, NKE], F32)
        nc.sync.dma_start(out=b1[:], in_=b1_e[:])
        wih = consts.tile([128, NKE, 4 * HID], BF16)
        nc.sync.dma_start(out=wih[:], in_=wih_e[:])
        w2t = consts.tile([128, NKH, NQ], BF16)
        nc.sync.dma_start(out=w2t[:], in_=w2t_e[:])
        cpp = consts.tile([128, NKH], F32)
        nc.sync.dma_start(out=cpp[:], in_=c_e[:])
        b2 = consts.tile([NQ, 1], F32)
        nc.sync.dma_start(out=b2[:], in_=b2_e[:])

        # gate type order: g first, then i (enables i*g), f (c_new), o (h).
        GATE_ORDER = (2, 0, 1, 3)

        # ---- main loop over batch tiles ----
        for t in range(NT):
            xt = xt_cur

            # MLP1: y1T[e] = relu(sum_c W1T[c,e].T @ xT[c] + b1[e])
            y1 = y1_p.tile([128, NKE, TILE], BF16, tag="y1", name="y1")
            for e in range(NKE):
                for s in range(TILE // 512):
                    yp = y1ps.tile([128, 512], F32, tag="yp", name="yp")
                    for c in range(NKX):
                        nc.tensor.matmul(
                            yp[:],
                            w1t[:, c, e * 128 : (e + 1) * 128],
                            xt[:, c, s * 512 : (s + 1) * 512],
                            start=(c == 0),
                            stop=(c == NKX - 1),
                        )
                    nc.vector.tensor_scalar(
                        y1[:, e, s * 512 : (s + 1) * 512],
                        yp[:],
                        b1[:, e : e + 1],
                        0.0,
                        op0=OP.add,
                        op1=OP.max,
                    )

            # prefetch next tile's inputs while PE chews on gates
            if t + 1 < NT:
                xt_cur = load_x(t + 1, nc.gpsimd)

            # gates + LSTM elementwise, interleaved per gate type
            gsb = [
                gate_p.tile([128, NKH, TILE], BF16, tag="gate", name=f"gate{gi}")
                for gi in range(4)
            ]
            ig = ig_p.tile([128, NKH, TILE], BF16, tag="ig", name="ig")
            cn = cn_p.tile([128, NKH, TILE], BF16, tag="cn", name="cn")
            th = ig_p.tile([128, NKH, TILE], BF16, tag="th", name="th")
            hsb = h_p.tile([128, NKH, TILE], BF16, tag="hsb", name="hsb")

            for gt in GATE_ORDER:
                for hh in range(NKH):
                    gp = gps.tile([128, TILE], F32, tag="gp", name="gp")
                    col = gt * HID + hh * 128
                    for s in range(TILE // 512):
                        for k in range(NKE):
                            nc.tensor.matmul(
                                gp[:, s * 512 : (s + 1) * 512],
                                wih[:, k, col : col + 128],
                                y1[:, k, s * 512 : (s + 1) * 512],
                                start=(k == 0),
                                stop=(k == NKE - 1),
                            )
                    nc.scalar.activation(
                        out=gsb[gt][:, hh, :],
                        in_=gp[:],
                        func=AF.Tanh if gt == 2 else AF.Sigmoid,
                        bias=gb[:, gt * NKH + hh : gt * NKH + hh + 1],
                    )
                    if gt == 0:
                        nc.vector.tensor_mul(
                            ig[:, hh, :], gsb[0][:, hh, :], gsb[2][:, hh, :]
                        )
                    elif gt == 1:
                        nc.vector.scalar_tensor_tensor(
                            out=cn[:, hh, :],
                            in0=gsb[1][:, hh, :],
                            scalar=cpp[:, hh : hh + 1],
                            in1=ig[:, hh, :],
                            op0=OP.mult,
                            op1=OP.add,
                        )
                        nc.scalar.activation(
                            out=th[:, hh, :], in_=cn[:, hh, :], func=AF.Tanh
                        )
                    elif gt == 3:
                        nc.vector.tensor_mul(
                            hsb[:, hh, :], gsb[3][:, hh, :], th[:, hh, :]
                        )

            # q = W2 @ h + b2, transposed, then X-bar back to natural layout
            qp = qps.tile([NQ, TILE], F32, tag="qp", name="qp")
            for s in range(TILE // 512):
                for k in range(NKH):
                    nc.tensor.matmul(
                        qp[:, s * 512 : (s + 1) * 512],
                        w2t[:, k, :],
                        hsb[:, k, s * 512 : (s + 1) * 512],
                        start=(k == 0),
                        stop=(k == NKH - 1),
                    )
            qt = qt_p.tile([NQ, TILE], BF16, tag="qt", name="qt")
            nc.scalar.activation(out=qt[:], in_=qp[:], func=AF.Identity, bias=b2[:, 0:1])
            qtr = qtr_p.tile([128, NU, NQ], BF16, tag="qtr", name="qtr")
            nc.sync.dma_start(out=qtr[:], in_=qt[:], transpose=True)
            qf = qf_p.tile([128, NU, NQ], F32, tag="qf", name="qf")
            nc.gpsimd.tensor_copy(qf[:], qtr[:])
            b0 = t * TILE
            nc.sync.dma_start(
                out=q_o[b0 : b0 + TILE, :].rearrange("(u p) d -> p u d", p=128),
                in_=qf[:],
            )

            if t == NT - 1:
                hc = out_p.tile([128, 2 * NKH], F32)
                nc.gpsimd.tensor_copy(
                    hc[:, 0:NKH], hsb[:, :, TILE - 1 : TILE].rearrange("p c o -> p (c o)")
                )
                nc.gpsimd.tensor_copy(
                    hc[:, NKH : 2 * NKH],
                    cn[:, :, TILE - 1 : TILE].rearrange("p c o -> p (c o)"),
                )
                nc.sync.dma_start(out=hc_o[:], in_=hc[:])

    nc.compile()
    return nc


def _bf16(a):
    return np.asarray(a).astype(ml_dtypes.bfloat16)


def _prep_consts(W1, b1, W_ih, b_ih, W_hh, b_hh, W2, b2, cell_state):
    w1t = np.zeros((XF, EMB), np.float32)
    w1t[: OBS + ACT] = np.asarray(W1).T          # [544, 512]
    w1t = _bf16(w1t.reshape(NKX, 128, EMB).transpose(1, 0, 2)).copy()
    wih = _bf16(np.asarray(W_ih).T.reshape(NKE, 128, 4 * HID).transpose(1, 0, 2)).copy()
    whht = _bf16(np.asarray(W_hh).T.reshape(NKH, 128, 4 * HID).transpose(1, 0, 2)).copy()
    w2t = _bf16(np.asarray(W2).T.reshape(NKH, 128, NQ).transpose(1, 0, 2)).copy()
    b1pp = np.asarray(b1, np.float32).reshape(NKE, 128).T.copy()
    bsumpp = (np.asarray(b_ih, np.float32) + np.asarray(b_hh, np.float32)).reshape(16, 128).T.copy()
    cpp = np.asarray(cell_state, np.float32).reshape(NKH, 128).T.copy()
    b2pp = np.asarray(b2, np.float32).reshape(NQ, 1).copy()
    return dict(w1t=w1t, wih=wih, whht=whht, w2t=w2t, b1pp=b1pp, bsumpp=bsumpp,
                cpp=cpp, b2pp=b2pp)


def kernel(observation, prev_action, hidden_state, cell_state,
           W1, b1, W_ih, b_ih, W_hh, b_hh, W2, b2):
    global _COMPILED
    if _COMPILED is None:
        _COMPILED = _build()
    nc = _COMPILED

    observation = np.asarray(observation, np.float32)
    prev_action = np.asarray(prev_action, np.float32)
    consts = _prep_consts(W1, b1, W_ih, b_ih, W_hh, b_hh, W2, b2, cell_state)
    hpp = np.asarray(hidden_state, np.float32).reshape(NKH, 128).T.copy()

    in_maps = []
    for i in range(NCORES):
        s = slice(i * BC, (i + 1) * BC)
        in_maps.append({
            "observation": np.ascontiguousarray(observation[s]),
            "prev_action": np.ascontiguousarray(prev_action[s]),
            "hpp": hpp,
            **consts,
        })

    res = run_bass_kernel_spmd(nc, in_maps, core_ids=list(range(NCORES)))
    q = np.concatenate([res.results[i]["q_out"] for i in range(NCORES)], axis=0)
    hc = res.results[NCORES - 1]["hc_out"]        # [128, 8]: hc[p, c] = v[c*128+p]
    h_last = np.ascontiguousarray(hc[:, 0:NKH].T).reshape(HID)
    c_last = np.ascontiguousarray(hc[:, NKH : 2 * NKH].T).reshape(HID)
    return q, h_last, c_last


# revision 24
# speedup vs baseline: 1.1515x; 1.1515x over previous
"""DRQN forward on 8 TRN2 NeuronCores, data-parallel over batch.

Reference computation (per batch row b):
    x      = concat(obs[b], act[b])                       [544]
    y1     = relu(W1 @ x + b1)                            [512]
    gates  = W_ih @ y1 + b_ih + (W_hh @ h + b_hh)         [2048]
    i,f,g,o = split(gates, 4); i,f,o = sigmoid; g = tanh
    c_new  = f * c + i * g                                [512]
    h_new  = o * tanh(c_new)                              [512]
    q      = W2 @ h_new + b2                              [32]
Returns (q [B,32], h_new[-1], c_new[-1]).

Device strategy (per core, batch shard of 4096 rows, 4 tiles of 1024):
  - activations kept transposed [feature, batch] in SBUF; the one real
    transpose (x) uses the DMA X-bar (bf16, one 3D-out instruction per
    128-row subtile) after an f32->bf16 cast on GpSimd.
  - all matmuls bf16 with fp32 PSUM accumulation.
  - gate bias (b_ih + b_hh + W_hh @ h) is per-partition in this layout and
    fuses into the ScalarE sigmoid/tanh ops; computed once on-device via
    DVE elementwise + reduce against a broadcast h.
  - gate types processed g,i,f,o so the LSTM elementwise work (DVE) can
    start per hid-chunk as soon as its inputs' activations land.
  - q computed transposed [32, batch], X-bar'd back, cast f32, stored
    in natural layout.
"""

import sys

sys.path.insert(0, "/opt/trn_rl_repo")

from contextlib import ExitStack

import numpy as np
import ml_dtypes

import concourse.bass as bass
import concourse.tile as tile
from concourse import bacc, mybir
from concourse.bass_utils import run_bass_kernel_spmd

BF16 = mybir.dt.bfloat16
F32 = mybir.dt.float32
AF = mybir.ActivationFunctionType
OP = mybir.AluOpType

OBS, ACT, EMB, HID, NQ = 512, 32, 512, 512, 32
B = 32768
NCORES = 8
BC = B // NCORES          # rows per core
TILE = 1024               # batch rows per main-loop tile
NT = BC // TILE           # main-loop tiles per core
NU = TILE // 128          # 128-row subtiles per tile
XF = 640                  # padded input feature dim (544 -> 5*128)
NKX = XF // 128           # K chunks for MLP1
NKE = EMB // 128          # K chunks over EMB
NKH = HID // 128          # K chunks over HID

_COMPILED = None


def _build():
    nc = bacc.Bacc(
        "TRN2", target_bir_lowering=False, debug=False,
        enable_asserts=False, num_devices=NCORES,
    )

    obs_e = nc.dram_tensor("observation", [BC, OBS], F32, kind="ExternalInput").ap()
    act_e = nc.dram_tensor("prev_action", [BC, ACT], F32, kind="ExternalInput").ap()
    hpp_e = nc.dram_tensor("hpp", [128, NKH], F32, kind="ExternalInput").ap()
    w1t_e = nc.dram_tensor("w1t", [128, NKX, EMB], BF16, kind="ExternalInput").ap()
    wih_e = nc.dram_tensor("wih", [128, NKE, 4 * HID], BF16, kind="ExternalInput").ap()
    whht_e = nc.dram_tensor("whht", [128, NKH, 4 * HID], BF16, kind="ExternalInput").ap()
    w2t_e = nc.dram_tensor("w2t", [128, NKH, NQ], BF16, kind="ExternalInput").ap()
    b1_e = nc.dram_tensor("b1pp", [128, NKE], F32, kind="ExternalInput").ap()
    bsum_e = nc.dram_tensor("bsumpp", [128, 16], F32, kind="ExternalInput").ap()
    c_e = nc.dram_tensor("cpp", [128, NKH], F32, kind="ExternalInput").ap()
    b2_e = nc.dram_tensor("b2pp", [NQ, 1], F32, kind="ExternalInput").ap()

    q_o = nc.dram_tensor("q_out", [BC, NQ], F32, kind="ExternalOutput").ap()
    # h_new[-1] and c_new[-1], stored [128, chunk] (hc[p, c] = v[c*128+p]);
    # the host de-permutes. Avoids a 512x4B-descriptor DMA in the tail.
    hc_o = nc.dram_tensor("hc_out", [128, 2 * NKH], F32, kind="ExternalOutput").ap()

    with tile.TileContext(nc) as tc, ExitStack() as ctx:
        consts = ctx.enter_context(tc.tile_pool(name="consts", bufs=1))
        obs_p = ctx.enter_context(tc.tile_pool(name="obs", bufs=4))
        act_p = ctx.enter_context(tc.tile_pool(name="act", bufs=4))
        xbf_p = ctx.enter_context(tc.tile_pool(name="xbf", bufs=2))
        xt_p = ctx.enter_context(tc.tile_pool(name="xt", bufs=2))
        y1_p = ctx.enter_context(tc.tile_pool(name="y1", bufs=2))
        gate_p = ctx.enter_context(tc.tile_pool(name="gate", bufs=4))
        ig_p = ctx.enter_context(tc.tile_pool(name="ig", bufs=2))
        cn_p = ctx.enter_context(tc.tile_pool(name="cn", bufs=2))
        h_p = ctx.enter_context(tc.tile_pool(name="hsb", bufs=2))
        qt_p = ctx.enter_context(tc.tile_pool(name="qt", bufs=2))
        qtr_p = ctx.enter_context(tc.tile_pool(name="qtr", bufs=2))
        qf_p = ctx.enter_context(tc.tile_pool(name="qf", bufs=2))
        out_p = ctx.enter_context(tc.tile_pool(name="outs", bufs=1))

        y1ps = ctx.enter_context(tc.tile_pool(name="y1ps", bufs=2, space="PSUM"))
        gps = ctx.enter_context(tc.tile_pool(name="gps", bufs=2, space="PSUM"))
        qps = ctx.enter_context(tc.tile_pool(name="qps", bufs=1, space="PSUM"))

        # ---- x load + cast + X-bar transpose for one tile. Three passes:
        # all loads, then all casts, then all X-bars — an X-bar emitted
        # between loads would stall the DMA queue on its cast dependency.
        # One xbf tile per 128-row subtile so each X-bar waits only on its
        # own casts. ----
        def load_x(t, cast_eng, xbar_engs=None):
            b0 = t * TILE
            xt = xt_p.tile([128, NKX, TILE], BF16, tag="xt", name="xt")
            obs_rs, act_rs, xbfs = [], [], []
            for u in range(NU):
                r0 = b0 + u * 128
                obs_r = obs_p.tile([128, OBS], F32, tag="obs", name="obs_r")
                nc.sync.dma_start(out=obs_r[:], in_=obs_e[r0 : r0 + 128, :])
                act_r = act_p.tile([128, ACT], F32, tag="act", name="act_r")
                nc.sync.dma_start(out=act_r[:], in_=act_e[r0 : r0 + 128, :])
                obs_rs.append(obs_r)
                act_rs.append(act_r)
            for u in range(NU):
                xbf = xbf_p.tile([128, XF], BF16, tag="xbf", name="xbf")
                cast_eng.tensor_copy(xbf[:, 0:OBS], obs_rs[u][:])
                cast_eng.tensor_copy(xbf[:, OBS : OBS + ACT], act_rs[u][:])
                cast_eng.memset(xbf[:, OBS + ACT : XF], 0.0)
                xbfs.append(xbf)
            engs = xbar_engs or (nc.sync,)
            for u in range(NU):
                engs[u % len(engs)].dma_start(
                    out=xt[:, :, u * 128 : (u + 1) * 128],
                    in_=xbfs[u][:],
                    transpose=True,
                )
            return xt

        # ---- preamble inputs for gb first, then tile-0 inputs, then the rest
        hppf = consts.tile([128, NKH], F32)
        nc.sync.dma_start(out=hppf[:], in_=hpp_e[:])
        whht = consts.tile([128, NKH, 4 * HID], BF16)
        nc.sync.dma_start(out=whht[:], in_=whht_e[:])
        bsum = consts.tile([128, 16], F32)
        nc.sync.dma_start(out=bsum[:], in_=bsum_e[:])
        gb = consts.tile([128, 16], F32)

        # gb[j*128+p] = (W_hh @ h)[j*128+p] + (b_ih+b_hh), via 64 tiny N=1
        # matmuls on the (otherwise idle at the head) TensorE
        hT = consts.tile([128, NKH], BF16)
        nc.vector.tensor_copy(hT[:], hppf[:])
        gbp = y1ps.tile([128, 16], F32, tag="yp", name="gbp")
        for j in range(16):
            for k in range(NKH):
                nc.tensor.matmul(
                    gbp[:, j : j + 1],
                    whht[:, k, j * 128 : (j + 1) * 128],
                    hT[:, k : k + 1],
                    start=(k == 0),
                    stop=(k == NKH - 1),
                )
        nc.vector.tensor_add(gb[:], gbp[:], bsum[:])

        # tile-0 input DMAs + casts on DVE (GpSimd casts are ~4x slower and
        # tile 0 is on the critical path); X-bars split across both HWDGE
        # engines (ScalarE is idle this early)
        xt_cur = load_x(0, nc.vector, xbar_engs=(nc.sync, nc.scalar))

        # ---- remaining constants ----
        w1t = consts.tile([128, NKX, EMB], BF16)
        nc.sync.dma_start(out=w1t[:], in_=w1t_e[:])
        b1 = consts.tile([128, NKE], F32)
        nc.sync.dma_start(out=b1[:], in_=b1_e[:])
        wih = consts.tile([128, NKE, 4 * HID], BF16)
        nc.sync.dma_start(out=wih[:], in_=wih_e[:])
        w2t = consts.tile([128, NKH, NQ], BF16)
        nc.sync.dma_start(out=w2t[:], in_=w2t_e[:])
        cpp = consts.tile([128, NKH], F32)
        nc.sync.dma_start(out=cpp[:], in_=c_e[:])
        b2 = consts.tile([NQ, 1], F32)
        nc.sync.dma_start(out=b2[:], in_=b2_e[:])

        # gate type order: g first, then i (enables i*g), f (c_new), o (h).
        GATE_ORDER = (2, 0, 1, 3)

        # ---- main loop over batch tiles ----
        for t in range(NT):
            xt = xt_cur

            # MLP1: y1T[e] = relu(sum_c W1T[c,e].T @ xT[c] + b1[e])
            y1 = y1_p.tile([128, NKE, TILE], BF16, tag="y1", name="y1")
            for e in range(NKE):
                for s in range(TILE // 512):
                    yp = y1ps.tile([128, 512], F32, tag="yp", name="yp")
                    for c in range(NKX):
                        nc.tensor.matmul(
                            yp[:],
                            w1t[:, c, e * 128 : (e + 1) * 128],
                            xt[:, c, s * 512 : (s + 1) * 512],
                            start=(c == 0),
                            stop=(c == NKX - 1),
                        )
                    nc.vector.tensor_scalar(
                        y1[:, e, s * 512 : (s + 1) * 512],
                        yp[:],
                        b1[:, e : e + 1],
                        0.0,
                        op0=OP.add,
                        op1=OP.max,
                    )

            # prefetch next tile's inputs while PE chews on gates
            if t + 1 < NT:
                xt_cur = load_x(t + 1, nc.gpsimd)

            # gates + LSTM elementwise, interleaved per gate type
            gsb = [
                gate_p.tile([128, NKH, TILE], BF16, tag="gate", name=f"gate{gi}")
                for gi in range(4)
            ]
            ig = ig_p.tile([128, NKH, TILE], BF16, tag="ig", name="ig")
            cn = cn_p.tile([128, NKH, TILE], BF16, tag="cn", name="cn")
            th = ig_p.tile([128, NKH, TILE], BF16, tag="th", name="th")
            hsb = h_p.tile([128, NKH, TILE], BF16, tag="hsb", name="hsb")

            for gt in GATE_ORDER:
                for hh in range(NKH):
                    gp = gps.tile([128, TILE], F32, tag="gp", name="gp")
                    col = gt * HID + hh * 128
                    for s in range(TILE // 512):
                        for k in range(NKE):
                            nc.tensor.matmul(
                                gp[:, s * 512 : (s + 1) * 512],
                                wih[:, k, col : col + 128],
                                y1[:, k, s * 512 : (s + 1) * 512],
                                start=(k == 0),
                                stop=(k == NKE - 1),
                            )
                    nc.scalar.activation(
                        out=gsb[gt][:, hh, :],
                        in_=gp[:],
                        func=AF.Tanh if gt == 2 else AF.Sigmoid,
                        bias=gb[:, gt * NKH + hh : gt * NKH + hh + 1],
                    )
                    if gt == 0:
                        nc.vector.tensor_mul(
                            ig[:, hh, :], gsb[0][:, hh, :], gsb[2][:, hh, :]
                        )
                    elif gt == 1:
                        nc.vector.scalar_tensor_tensor(
                            out=cn[:, hh, :],
                            in0=gsb[1][:, hh, :],
                            scalar=cpp[:, hh : hh + 1],
                            in1=ig[:, hh, :],
                            op0=OP.mult,
                            op1=OP.add,
                        )
                        nc.scalar.activation(
                            out=th[:, hh, :], in_=cn[:, hh, :], func=AF.Tanh
                        )
                    elif gt == 3:
                        nc.vector.tensor_mul(
                            hsb[:, hh, :], gsb[3][:, hh, :], th[:, hh, :]
                        )

            # q = W2 @ h + b2, transposed, then X-bar back to natural layout
            qp = qps.tile([NQ, TILE], F32, tag="qp", name="qp")
            for s in range(TILE // 512):
                for k in range(NKH):
                    nc.tensor.matmul(
                        qp[:, s * 512 : (s + 1) * 512],
                        w2t[:, k, :],
                        hsb[:, k, s * 512 : (s + 1) * 512],
                        start=(k == 0),
                        stop=(k == NKH - 1),
                    )
            qt = qt_p.tile([NQ, TILE], BF16, tag="qt", name="qt")
            nc.scalar.activation(out=qt[:], in_=qp[:], func=AF.Identity, bias=b2[:, 0:1])
            qtr = qtr_p.tile([128, NU, NQ], BF16, tag="qtr", name="qtr")
            nc.sync.dma_start(out=qtr[:], in_=qt[:], transpose=True)
            qf = qf_p.tile([128, NU, NQ], F32, tag="qf", name="qf")
            nc.gpsimd.tensor_copy(qf[:], qtr[:])
            b0 = t * TILE
            nc.sync.dma_start(
                out=q_o[b0 : b0 + TILE, :].rearrange("(u p) d -> p u d", p=128),
                in_=qf[:],
            )

            if t == NT - 1:
                hc = out_p.tile([128, 2 * NKH], F32)
                nc.gpsimd.tensor_copy(
                    hc[:, 0:NKH], hsb[:, :, TILE - 1 : TILE].rearrange("p c o -> p (c o)")
                )
                nc.gpsimd.tensor_copy(
                    hc[:, NKH : 2 * NKH],
                    cn[:, :, TILE - 1 : TILE].rearrange("p c o -> p (c o)"),
                )
                nc.sync.dma_start(out=hc_o[:], in_=hc[:])

    nc.compile()
    return nc


def _bf16(a):
    return np.asarray(a).astype(ml_dtypes.bfloat16)


def _prep_consts(W1, b1, W_ih, b_ih, W_hh, b_hh, W2, b2, cell_state):
    w1t = np.zeros((XF, EMB), np.float32)
    w1t[: OBS + ACT] = np.asarray(W1).T          # [544, 512]
    w1t = _bf16(w1t.reshape(NKX, 128, EMB).transpose(1, 0, 2)).copy()
    wih = _bf16(np.asarray(W_ih).T.reshape(NKE, 128, 4 * HID).transpose(1, 0, 2)).copy()
    whht = _bf16(np.asarray(W_hh).T.reshape(NKH, 128, 4 * HID).transpose(1, 0, 2)).copy()
    w2t = _bf16(np.asarray(W2).T.reshape(NKH, 128, NQ).transpose(1, 0, 2)).copy()
    b1pp = np.asarray(b1, np.float32).reshape(NKE, 128).T.copy()
    bsumpp = (np.asarray(b_ih, np.float32) + np.asarray(b_hh, np.float32)).reshape(16, 128).T.copy()
    cpp = np.asarray(cell_state, np.float32).reshape(NKH, 128).T.copy()
    b2pp = np.asarray(b2, np.float32).reshape(NQ, 1).copy()
    return dict(w1t=w1t, wih=wih, whht=whht, w2t=w2t, b1pp=b1pp, bsumpp=bsumpp,
                cpp=cpp, b2pp=b2pp)


def kernel(observation, prev_action, hidden_state, cell_state,
           W1, b1, W_ih, b_ih, W_hh, b_hh, W2, b2):
    global _COMPILED
    if _COMPILED is None:
        _COMPILED = _build()
    nc = _COMPILED

    observation = np.asarray(observation, np.float32)
    prev_action = np.asarray(prev_action, np.float32)
    consts = _prep_consts(W1, b1, W_ih, b_ih, W_hh, b_hh, W2, b2, cell_state)
    hpp = np.asarray(hidden_state, np.float32).reshape(NKH, 128).T.copy()

    in_maps = []
    for i in range(NCORES):
        s = slice(i * BC, (i + 1) * BC)
        in_maps.append({
            "observation": np.ascontiguousarray(observation[s]),
            "prev_action": np.ascontiguousarray(prev_action[s]),
            "hpp": hpp,
            **consts,
        })

    res = run_bass_kernel_spmd(nc, in_maps, core_ids=list(range(NCORES)))
    q = np.concatenate([res.results[i]["q_out"] for i in range(NCORES)], axis=0)
    hc = res.results[NCORES - 1]["hc_out"]        # [128, 8]: hc[p, c] = v[c*128+p]
    h_last = np.ascontiguousarray(hc[:, 0:NKH].T).reshape(HID)
    c_last = np.ascontiguousarray(hc[:, NKH : 2 * NKH].T).reshape(HID)
    return q, h_last, c_last


# revision 28
# speedup vs baseline: 1.3074x; 1.1354x over previous
"""DRQN forward on 8 TRN2 NeuronCores, data-parallel over batch.

Reference computation (per batch row b):
    x      = concat(obs[b], act[b])                       [544]
    y1     = relu(W1 @ x + b1)                            [512]
    gates  = W_ih @ y1 + b_ih + (W_hh @ h + b_hh)         [2048]
    i,f,g,o = split(gates, 4); i,f,o = sigmoid; g = tanh
    c_new  = f * c + i * g                                [512]
    h_new  = o * tanh(c_new)                              [512]
    q      = W2 @ h_new + b2                              [32]
Returns (q [B,32], h_new[-1], c_new[-1]).

Device strategy (per core, batch shard of 4096 rows, 4 tiles of 1024):
  - activations kept transposed [feature, batch] in SBUF; the one real
    transpose (x) uses the DMA X-bar (bf16, one 3D-out instruction per
    128-row subtile) after an f32->bf16 cast on GpSimd.
  - all matmuls bf16 with fp32 PSUM accumulation.
  - gate bias (b_ih + b_hh + W_hh @ h) is per-partition in this layout and
    fuses into the ScalarE sigmoid/tanh ops; computed once on-device via
    DVE elementwise + reduce against a broadcast h.
  - gate types processed g,i,f,o so the LSTM elementwise work (DVE) can
    start per hid-chunk as soon as its inputs' activations land.
  - q computed transposed [32, batch], X-bar'd back, cast f32, stored
    in natural layout.
"""

import sys

sys.path.insert(0, "/opt/trn_rl_repo")

from contextlib import ExitStack

import numpy as np
import ml_dtypes

import concourse.bass as bass
import concourse.tile as tile
from concourse import bacc, mybir
from concourse.bass_utils import run_bass_kernel_spmd

BF16 = mybir.dt.bfloat16
F32 = mybir.dt.float32
AF = mybir.ActivationFunctionType
OP = mybir.AluOpType

OBS, ACT, EMB, HID, NQ = 512, 32, 512, 512, 32
B = 32768
NCORES = 8
BC = B // NCORES          # rows per core
TILE = 1024               # batch rows per main-loop tile
NT = BC // TILE           # main-loop tiles per core
NU = TILE // 128          # 128-row subtiles per tile
XF = 640                  # padded input feature dim (544 -> 5*128)
NKX = XF // 128           # K chunks for MLP1
NKE = EMB // 128          # K chunks over EMB
NKH = HID // 128          # K chunks over HID

_COMPILED = None


def _build():
    nc = bacc.Bacc(
        "TRN2", target_bir_lowering=False, debug=False,
        enable_asserts=False, num_devices=NCORES,
    )

    obs_e = nc.dram_tensor("observation", [BC, OBS], F32, kind="ExternalInput").ap()
    act_e = nc.dram_tensor("prev_action", [BC, ACT], F32, kind="ExternalInput").ap()
    hpp_e = nc.dram_tensor("hpp", [128, NKH], F32, kind="ExternalInput").ap()
    w1t_e = nc.dram_tensor("w1t", [128, NKX, EMB], BF16, kind="ExternalInput").ap()
    wih_e = nc.dram_tensor("wih", [128, NKE, 4 * HID], BF16, kind="ExternalInput").ap()
    whht_e = nc.dram_tensor("whht", [128, NKH, 4 * HID], BF16, kind="ExternalInput").ap()
    w2t_e = nc.dram_tensor("w2t", [128, NKH, NQ], BF16, kind="ExternalInput").ap()
    b1_e = nc.dram_tensor("b1pp", [128, NKE], F32, kind="ExternalInput").ap()
    bsum_e = nc.dram_tensor("bsumpp", [128, 16], F32, kind="ExternalInput").ap()
    c_e = nc.dram_tensor("cpp", [128, NKH], F32, kind="ExternalInput").ap()
    b2_e = nc.dram_tensor("b2pp", [NQ, 1], F32, kind="ExternalInput").ap()

    q_o = nc.dram_tensor("q_out", [BC, NQ], F32, kind="ExternalOutput").ap()
    # h_new[-1] and c_new[-1], stored [128, chunk] (hc[p, c] = v[c*128+p]);
    # the host de-permutes. Avoids a 512x4B-descriptor DMA in the tail.
    hc_o = nc.dram_tensor("hc_out", [128, 2 * NKH], F32, kind="ExternalOutput").ap()

    with tile.TileContext(nc) as tc, ExitStack() as ctx:
        consts = ctx.enter_context(tc.tile_pool(name="consts", bufs=1))
        obs_p = ctx.enter_context(tc.tile_pool(name="obs", bufs=3))
        act_p = ctx.enter_context(tc.tile_pool(name="act", bufs=2))
        xbf_p = ctx.enter_context(tc.tile_pool(name="xbf", bufs=2))
        xt_p = ctx.enter_context(tc.tile_pool(name="xt", bufs=2))
        y1_p = ctx.enter_context(tc.tile_pool(name="y1", bufs=2))
        gate_p = ctx.enter_context(tc.tile_pool(name="gate", bufs=4))
        ig_p = ctx.enter_context(tc.tile_pool(name="ig", bufs=1))
        cn_p = ctx.enter_context(tc.tile_pool(name="cn", bufs=1))
        h_p = ctx.enter_context(tc.tile_pool(name="hsb", bufs=2))
        qt_p = ctx.enter_context(tc.tile_pool(name="qt", bufs=2))
        qtr_p = ctx.enter_context(tc.tile_pool(name="qtr", bufs=2))
        qf_p = ctx.enter_context(tc.tile_pool(name="qf", bufs=2))
        out_p = ctx.enter_context(tc.tile_pool(name="outs", bufs=1))

        y1ps = ctx.enter_context(tc.tile_pool(name="y1ps", bufs=2, space="PSUM"))
        gps = ctx.enter_context(tc.tile_pool(name="gps", bufs=2, space="PSUM"))
        qps = ctx.enter_context(tc.tile_pool(name="qps", bufs=1, space="PSUM"))

        # ---- x load + cast + X-bar transpose for one tile.
        # Loads use a rows-per-partition layout: partition p holds the NU
        # consecutive batch rows b0+p*NU .. b0+p*NU+NU-1, giving one DMA per
        # tile with NU*2KB contiguous per partition (128 fat descriptors
        # instead of 2048 thin ones). Batch order inside the tile becomes
        # col = j*128+p <-> row b0+p*NU+j, which is irrelevant for the math;
        # the q store and h/c extraction use the matching layout.
        def load_x(t, cast_eng, xbar_engs=None):
            b0 = t * TILE
            obs_ap = obs_e[b0 : b0 + TILE, :].rearrange("(p j) d -> p j d", p=128)
            xbf = xbf_p.tile([128, NU, XF], BF16, tag="xbf", name="xbf")
            for hf in range(2):
                js = slice(hf * (NU // 2), (hf + 1) * (NU // 2))
                obs_r = obs_p.tile([128, NU // 2, OBS], F32, tag="obs", name="obs_r")
                nc.sync.dma_start(out=obs_r[:], in_=obs_ap[:, js, :])
                cast_eng.tensor_copy(xbf[:, js, 0:OBS], obs_r[:])
            act_r = act_p.tile([128, NU, ACT], F32, tag="act", name="act_r")
            nc.sync.dma_start(
                out=act_r[:],
                in_=act_e[b0 : b0 + TILE, :].rearrange("(p j) d -> p j d", p=128),
            )
            cast_eng.tensor_copy(xbf[:, :, OBS : OBS + ACT], act_r[:])
            cast_eng.memset(xbf[:, :, OBS + ACT : XF], 0.0)
            xt = xt_p.tile([128, NKX, TILE], BF16, tag="xt", name="xt")
            engs = xbar_engs or (nc.sync,)
            for u in range(NU):
                engs[u % len(engs)].dma_start(
                    out=xt[:, :, u * 128 : (u + 1) * 128],
                    in_=xbf[:, u, :],
                    transpose=True,
                )
            return xt

        # ---- preamble inputs for gb first, then tile-0 inputs, then the rest
        hppf = consts.tile([128, NKH], F32)
        nc.sync.dma_start(out=hppf[:], in_=hpp_e[:])
        whht = consts.tile([128, NKH, 4 * HID], BF16)
        nc.sync.dma_start(out=whht[:], in_=whht_e[:])
        bsum = consts.tile([128, 16], F32)
        nc.sync.dma_start(out=bsum[:], in_=bsum_e[:])
        gb = consts.tile([128, 16], F32)

        # gb[j*128+p] = (W_hh @ h)[j*128+p] + (b_ih+b_hh), via 64 tiny N=1
        # matmuls on the (otherwise idle at the head) TensorE
        hT = consts.tile([128, NKH], BF16)
        nc.vector.tensor_copy(hT[:], hppf[:])
        gbp = y1ps.tile([128, 16], F32, tag="yp", name="gbp")
        for j in range(16):
            for k in range(NKH):
                nc.tensor.matmul(
                    gbp[:, j : j + 1],
                    whht[:, k, j * 128 : (j + 1) * 128],
                    hT[:, k : k + 1],
                    start=(k == 0),
                    stop=(k == NKH - 1),
                )
        nc.vector.tensor_add(gb[:], gbp[:], bsum[:])

        # tile-0 input DMAs + casts on DVE (GpSimd casts are ~4x slower and
        # tile 0 is on the critical path); X-bars split across both HWDGE
        # engines (ScalarE is idle this early)
        xt_cur = load_x(0, nc.vector, xbar_engs=(nc.sync, nc.scalar))

        # ---- remaining constants ----
        w1t = consts.tile([128, NKX, EMB], BF16)
        nc.sync.dma_start(out=w1t[:], in_=w1t_e[:])
        b1 = consts.tile([128, NKE], F32)
        nc.sync.dma_start(out=b1[:], in_=b1_e[:])
        wih = consts.tile([128, NKE, 4 * HID], BF16)
        nc.sync.dma_start(out=wih[:], in_=wih_e[:])
        w2t = consts.tile([128, NKH, NQ], BF16)
        nc.sync.dma_start(out=w2t[:], in_=w2t_e[:])
        cpp = consts.tile([128, NKH], F32)
        nc.sync.dma_start(out=cpp[:], in_=c_e[:])
        b2 = consts.tile([NQ, 1], F32)
        nc.sync.dma_start(out=b2[:], in_=b2_e[:])

        # gate type order: g first, then i (enables i*g), f (c_new), o (h).
        GATE_ORDER = (2, 0, 1, 3)

        # ---- main loop over batch tiles ----
        for t in range(NT):
            xt = xt_cur

            # MLP1: y1T[e] = relu(sum_c W1T[c,e].T @ xT[c] + b1[e])
            y1 = y1_p.tile([128, NKE, TILE], BF16, tag="y1", name="y1")
            for e in range(NKE):
                for s in range(TILE // 512):
                    yp = y1ps.tile([128, 512], F32, tag="yp", name="yp")
                    for c in range(NKX):
                        nc.tensor.matmul(
                            yp[:],
                            w1t[:, c, e * 128 : (e + 1) * 128],
                            xt[:, c, s * 512 : (s + 1) * 512],
                            start=(c == 0),
                            stop=(c == NKX - 1),
                        )
                    nc.vector.tensor_scalar(
                        y1[:, e, s * 512 : (s + 1) * 512],
                        yp[:],
                        b1[:, e : e + 1],
                        0.0,
                        op0=OP.add,
                        op1=OP.max,
                    )

            # prefetch next tile's inputs while PE chews on gates
            if t + 1 < NT:
                xt_cur = load_x(t + 1, nc.gpsimd)

            # gates + LSTM elementwise, interleaved per gate type
            gsb = [
                gate_p.tile([128, NKH, TILE], BF16, tag="gate", name=f"gate{gi}")
                for gi in range(4)
            ]
            ig = ig_p.tile([128, NKH, TILE], BF16, tag="ig", name="ig")
            cn = cn_p.tile([128, NKH, TILE], BF16, tag="cn", name="cn")
            th = ig_p.tile([128, NKH, TILE], BF16, tag="th", name="th")
            hsb = h_p.tile([128, NKH, TILE], BF16, tag="hsb", name="hsb")

            for gt in GATE_ORDER:
                for hh in range(NKH):
                    gp = gps.tile([128, TILE], F32, tag="gp", name="gp")
                    col = gt * HID + hh * 128
                    for s in range(TILE // 512):
                        for k in range(NKE):
                            nc.tensor.matmul(
                                gp[:, s * 512 : (s + 1) * 512],
                                wih[:, k, col : col + 128],
                                y1[:, k, s * 512 : (s + 1) * 512],
                                start=(k == 0),
                                stop=(k == NKE - 1),
                            )
                    nc.scalar.activation(
                        out=gsb[gt][:, hh, :],
                        in_=gp[:],
                        func=AF.Tanh if gt == 2 else AF.Sigmoid,
                        bias=gb[:, gt * NKH + hh : gt * NKH + hh + 1],
                    )
                    if gt == 0:
                        nc.vector.tensor_mul(
                            ig[:, hh, :], gsb[0][:, hh, :], gsb[2][:, hh, :]
                        )
                    elif gt == 1:
                        nc.vector.scalar_tensor_tensor(
                            out=cn[:, hh, :],
                            in0=gsb[1][:, hh, :],
                            scalar=cpp[:, hh : hh + 1],
                            in1=ig[:, hh, :],
                            op0=OP.mult,
                            op1=OP.add,
                        )
                        nc.scalar.activation(
                            out=th[:, hh, :], in_=cn[:, hh, :], func=AF.Tanh
                        )
                    elif gt == 3:
                        nc.vector.tensor_mul(
                            hsb[:, hh, :], gsb[3][:, hh, :], th[:, hh, :]
                        )

            # q = W2 @ h + b2, transposed, then X-bar back to natural layout
            qp = qps.tile([NQ, TILE], F32, tag="qp", name="qp")
            for s in range(TILE // 512):
                for k in range(NKH):
                    nc.tensor.matmul(
                        qp[:, s * 512 : (s + 1) * 512],
                        w2t[:, k, :],
                        hsb[:, k, s * 512 : (s + 1) * 512],
                        start=(k == 0),
                        stop=(k == NKH - 1),
                    )
            qt = qt_p.tile([NQ, TILE], BF16, tag="qt", name="qt")
            nc.scalar.activation(out=qt[:], in_=qp[:], func=AF.Identity, bias=b2[:, 0:1])
            qtr = qtr_p.tile([128, NU, NQ], BF16, tag="qtr", name="qtr")
            nc.sync.dma_start(out=qtr[:], in_=qt[:], transpose=True)
            qf = qf_p.tile([128, NU, NQ], F32, tag="qf", name="qf")
            nc.gpsimd.tensor_copy(qf[:], qtr[:])
            b0 = t * TILE
            nc.sync.dma_start(
                out=q_o[b0 : b0 + TILE, :].rearrange("(p j) d -> p j d", p=128),
                in_=qf[:],
            )

            if t == NT - 1:
                hc = out_p.tile([128, 2 * NKH], F32)
                nc.gpsimd.tensor_copy(
                    hc[:, 0:NKH], hsb[:, :, TILE - 1 : TILE].rearrange("p c o -> p (c o)")
                )
                nc.gpsimd.tensor_copy(
                    hc[:, NKH : 2 * NKH],
                    cn[:, :, TILE - 1 : TILE].rearrange("p c o -> p (c o)"),
                )
                nc.sync.dma_start(out=hc_o[:], in_=hc[:])

    nc.compile()
    return nc


def _bf16(a):
    return np.asarray(a).astype(ml_dtypes.bfloat16)


def _prep_consts(W1, b1, W_ih, b_ih, W_hh, b_hh, W2, b2, cell_state):
    w1t = np.zeros((XF, EMB), np.float32)
    w1t[: OBS + ACT] = np.asarray(W1).T          # [544, 512]
    w1t = _bf16(w1t.reshape(NKX, 128, EMB).transpose(1, 0, 2)).copy()
    wih = _bf16(np.asarray(W_ih).T.reshape(NKE, 128, 4 * HID).transpose(1, 0, 2)).copy()
    whht = _bf16(np.asarray(W_hh).T.reshape(NKH, 128, 4 * HID).transpose(1, 0, 2)).copy()
    w2t = _bf16(np.asarray(W2).T.reshape(NKH, 128, NQ).transpose(1, 0, 2)).copy()
    b1pp = np.asarray(b1, np.float32).reshape(NKE, 128).T.copy()
    bsumpp = (np.asarray(b_ih, np.float32) + np.asarray(b_hh, np.float32)).reshape(16, 128).T.copy()
    cpp = np.asarray(cell_state, np.float32).reshape(NKH, 128).T.copy()
    b2pp = np.asarray(b2, np.float32).reshape(NQ, 1).copy()
    return dict(w1t=w1t, wih=wih, whht=whht, w2t=w2t, b1pp=b1pp, bsumpp=bsumpp,
                cpp=cpp, b2pp=b2pp)


def kernel(observation, prev_action, hidden_state, cell_state,
           W1, b1, W_ih, b_ih, W_hh, b_hh, W2, b2):
    global _COMPILED
    if _COMPILED is None:
        _COMPILED = _build()
    nc = _COMPILED

    observation = np.asarray(observation, np.float32)
    prev_action = np.asarray(prev_action, np.float32)
    consts = _prep_consts(W1, b1, W_ih, b_ih, W_hh, b_hh, W2, b2, cell_state)
    hpp = np.asarray(hidden_state, np.float32).reshape(NKH, 128).T.copy()

    in_maps = []
    for i in range(NCORES):
        s = slice(i * BC, (i + 1) * BC)
        in_maps.append({
            "observation": np.ascontiguousarray(observation[s]),
            "prev_action": np.ascontiguousarray(prev_action[s]),
            "hpp": hpp,
            **consts,
        })

    res = run_bass_kernel_spmd(nc, in_maps, core_ids=list(range(NCORES)))
    q = np.concatenate([res.results[i]["q_out"] for i in range(NCORES)], axis=0)
    hc = res.results[NCORES - 1]["hc_out"]        # [128, 8]: hc[p, c] = v[c*128+p]
    h_last = np.ascontiguousarray(hc[:, 0:NKH].T).reshape(HID)
    c_last = np.ascontiguousarray(hc[:, NKH : 2 * NKH].T).reshape(HID)
    return q, h_last, c_last
